# revision 46
# baseline (speedup 1.0000x reference)
"""NeuralCDE RK4 solver as a Bass/Tile kernel on 8 Trainium2 cores.

Data-parallel over batch: B=1024 -> 128 rows per core. The 127-step RK4
scan is fully unrolled. Everything lives in T layout (features on
partitions, batch on the free dim); per stage the critical chain is:

    relu (ACT): hS[128m,128b] = relu(h_psum + bias1(t))      (t folded in bias)
    mm2T (PE) : f_ps[128hc,4*128b] = W2chunk_j.T @ hS        (4 MMs, one bank)
    tanh (ACT): fS = tanh(f_ps)                              (b2 == 0)
    mul  (DVE): u = fS * dxrep(step,cls)                     (dx broadcast over j)
    V    (PE) : h_psum[s+1] += (S_j@W1z*alpha).T @ u_j       (4 MMs; k never
                materialized -- feeds next stage's mm1 directly)
    S    (PE) : accP[64,128b] += S_j.T @ u_j                 (RK4 sum in PSUM)

Off the chain: one z-part matmul per stage (W1z.T @ z), one DVE op per
step for z_{i+1} = z + accP/6, and streamed DMAs for the host-built
dxrep tables (dX/dt values pre-replicated over the 16 h-groups, RK4
stage weights folded in).
"""

import numpy as np

import concourse.bacc as bacc
import concourse.bass as bass
import concourse.mybir as mybir
from concourse.tile import TileContext
from concourse.bass_utils import run_bass_kernel_spmd

F32 = mybir.dt.float32
FP16 = mybir.dt.float16
AF = mybir.ActivationFunctionType

B = 1024
L = 128
C_IN = 8
HID = 64
MLP_H = 128
NSTEP = L - 1  # 127
NCORES = 8
BL = B // NCORES  # 128 batch rows per core
NF = HID * C_IN  # 512

_CACHE: dict = {}


def _flags():
    import os

    return (
        os.environ.get("K_WARM", "1") == "1",
        int(os.environ.get("K_DXRING", "16")),
        int(os.environ.get("K_NFILL", "12")),
        os.environ.get("K_RELU_DVE", "0") == "1",
        os.environ.get("K_DX4", "0") == "1",
        int(os.environ.get("K_NF2", "0")),  # fillers after V (post-V window)
        int(os.environ.get("K_FW", "256")),  # filler free width
        os.environ.get("K_SAV", "0") == "1",  # pend-S after V
    )


def _build(nstep: int, with_b2: bool):
    import sys
    import time as _time

    warm, dxring, nfill, relu_dve, dx4, nf2, fw, sav = _flags()
    t0 = _time.time()
    nc = bacc.Bacc()
    DXW = 4 * BL if dx4 else BL
    dx_in = nc.dram_tensor("dxr", [128, nstep * 3 * DXW], FP16, kind="ExternalInput")
    b1_in = nc.dram_tensor("bias1", [MLP_H, nstep * 3], F32, kind="ExternalInput")
    w1z_in = nc.dram_tensor("w1z", [HID, MLP_H], F32, kind="ExternalInput")
    w1zh_in = nc.dram_tensor("w1zh", [HID, MLP_H], FP16, kind="ExternalInput")
    w2_in = nc.dram_tensor("w2", [MLP_H, NF], FP16, kind="ExternalInput")
    v_in = nc.dram_tensor("vtab", [128, 2 * NF], FP16, kind="ExternalInput")
    s_in = nc.dram_tensor("spat", [128, 4 * HID], FP16, kind="ExternalInput")
    b2_in = nc.dram_tensor("b2t", [128, 4], F32, kind="ExternalInput")
    z0t_in = nc.dram_tensor("z0t", [HID, BL], F32, kind="ExternalInput")
    zs_out = nc.dram_tensor(
        "zs", [HID, (nstep + 1) * BL], F32, kind="ExternalOutput"
    )

    CLS = (0, 1, 1, 2)
    # V-table variant per stage transition s -> s+1 (alpha 0.5, 0.25, 0.5)
    VVAR = (0, 1, 0)

    with TileContext(nc) as tc:
        with (
            tc.tile_pool(name="const", bufs=1) as cp,
            tc.tile_pool(name="zst", bufs=1) as zp,
            tc.tile_pool(name="dx", bufs=dxring) as dxp,
            tc.tile_pool(name="hs", bufs=3) as hp,
            tc.tile_pool(name="fs", bufs=2) as fsp,
            tc.tile_pool(name="us", bufs=3) as up,
            tc.tile_pool(name="zh", bufs=2) as zhp,
            tc.tile_pool(name="ph", bufs=4, space="PSUM") as ph,
            tc.tile_pool(name="pf", bufs=2, space="PSUM") as pf,
            tc.tile_pool(name="pacc", bufs=1, space="PSUM") as pacc,
            tc.tile_pool(name="pfill", bufs=1, space="PSUM") as pfill,
        ):
            b1S = cp.tile([MLP_H, nstep * 3], F32)
            w1zS = cp.tile([HID, MLP_H], F32)
            w1zhS = cp.tile([HID, MLP_H], FP16)
            w2S = cp.tile([MLP_H, NF], FP16)
            vS = cp.tile([128, 2 * NF], FP16)
            sS = cp.tile([128, 4 * HID], FP16)
            b2S = cp.tile([128, 4], F32)
            zall = zp.tile([HID, (nstep + 1) * BL], F32)

            nc.sync.dma_start(out=b1S[:], in_=b1_in[:])
            nc.sync.dma_start(out=w1zS[:], in_=w1z_in[:])
            nc.sync.dma_start(out=w1zhS[:], in_=w1zh_in[:])
            nc.sync.dma_start(out=w2S[:], in_=w2_in[:])
            nc.sync.dma_start(out=vS[:], in_=v_in[:])
            nc.sync.dma_start(out=sS[:], in_=s_in[:])
            nc.sync.dma_start(out=b2S[:], in_=b2_in[:])
            nc.sync.dma_start(out=zall[:, 0:BL], in_=z0t_in[:])
            nc.sync.dma_start(out=zs_out[:, 0:BL], in_=z0t_in[:].bitcast(F32))

            if warm:
                wtL = cp.tile([128, MLP_H], FP16, name="wtL")
                wtR = cp.tile([128, fw], FP16, name="wtR")
                nc.vector.memset(wtL[:], 0.0)
                nc.vector.memset(wtR[:], 0.0)
                wp = pfill.tile([128, NF], F32, tag="fl", name="wp")

            def fill(n):
                if warm:
                    for _f in range(n):
                        nc.tensor.matmul(
                            wp[:, 0:fw], lhsT=wtL[:], rhs=wtR[:],
                            start=True, stop=True,
                        )

            def zh_copy(z_ap):
                t = zhp.tile([HID, BL], FP16, tag="zh", name="zh")
                nc.vector.tensor_copy(t[:], z_ap)
                return t

            def h_group(znext_ap):
                """Open a step's 4 h PSUM tiles (one bank each -- start=True
                clears has_written bank-wide, so stages can't share a bank):
                z-part matmuls. Tile 0 (stage 0) is complete on its own;
                tiles 1..3 get V-matmul accumulation during stages 0..2.
                """
                tiles = []
                for s in range(4):
                    t = ph.tile([MLP_H, BL], F32, tag="hps", name=f"hps{s}")
                    nc.tensor.matmul(
                        t[:],
                        lhsT=w1zhS[:],
                        rhs=znext_ap,
                        start=True,
                        stop=(s == 0),
                        skip_group_check=True,
                    )
                    tiles.append(t)
                return tiles

            hT = h_group(zh_copy(zall[:, 0:BL])[:])

            for step in range(nstep):
                dxS = dxp.tile([128, 3 * DXW], FP16, tag="dx", name="dx")
                nc.sync.dma_start(
                    out=dxS[:], in_=dx_in[:, step * 3 * DXW : (step + 1) * 3 * DXW]
                )
                zT = zall[:, step * BL : (step + 1) * BL]
                accP = pacc.tile([HID, BL], F32, tag="acc", name="acc")
                pend = None
                for s in range(4):
                    col = step * 3 + CLS[s]
                    hS = hp.tile([MLP_H, BL], FP16, tag="hs", name="hs")
                    if relu_dve:
                        nc.vector.tensor_scalar(
                            hS[:],
                            hT[s][:],
                            b1S[:, col : col + 1],
                            0.0,
                            op0=mybir.AluOpType.add,
                            op1=mybir.AluOpType.max,
                        )
                    else:
                        nc.scalar.activation(
                            hS[:],
                            hT[s][:],
                            AF.Relu,
                            bias=b1S[:, col : col + 1],
                        )
                    f_ps = pf.tile([128, NF], F32, tag="fps", name="fps")
                    for j in range(4):
                        nc.tensor.matmul(
                            f_ps[:, j * BL : (j + 1) * BL],
                            lhsT=w2S[:, j * MLP_H : (j + 1) * MLP_H],
                            rhs=hS[:],
                            start=True,
                            stop=True,
                        )

                    def s_mms(pu, ps):
                        for j in range(4):
                            nc.tensor.matmul(
                                accP[:],
                                lhsT=sS[:, j * HID : (j + 1) * HID],
                                rhs=pu[:, j * BL : (j + 1) * BL],
                                start=(ps == 0 and j == 0),
                                stop=False,
                                skip_group_check=True,
                            )

                    if pend is not None and (not sav or s == 3):
                        pu, ps = pend
                        pend = None
                        s_mms(pu, ps)
                    fill(nfill)
                    fS = fsp.tile([128, NF], FP16, tag="fs", name="fs")
                    if with_b2:
                        for j in range(4):
                            nc.scalar.activation(
                                fS[:, j * BL : (j + 1) * BL],
                                f_ps[:, j * BL : (j + 1) * BL],
                                AF.Tanh,
                                bias=b2S[:, j : j + 1],
                            )
                    else:
                        nc.scalar.activation(fS[:], f_ps[:], AF.Tanh)
                    u = up.tile([128, NF], FP16, tag="u", name="u")
                    if dx4:
                        nc.vector.tensor_tensor(
                            out=u[:],
                            in0=fS[:],
                            in1=dxS[:, CLS[s] * NF : (CLS[s] + 1) * NF],
                            op=mybir.AluOpType.mult,
                        )
                    else:
                        u3 = u[:].rearrange("p (j b) -> p j b", j=4)
                        f3 = fS[:].rearrange("p (j b) -> p j b", j=4)
                        dxv = (
                            dxS[:, CLS[s] * BL : (CLS[s] + 1) * BL]
                            .unsqueeze(1)
                            .broadcast_to((128, 4, BL))
                        )
                        nc.vector.tensor_tensor(
                            out=u3, in0=f3, in1=dxv, op=mybir.AluOpType.mult
                        )
                    if s < 3:
                        voff = VVAR[s] * NF
                        for j in range(4):
                            nc.tensor.matmul(
                                hT[s + 1][:],
                                lhsT=vS[:, voff + j * MLP_H : voff + (j + 1) * MLP_H],
                                rhs=u[:, j * BL : (j + 1) * BL],
                                start=False,
                                stop=(j == 3),
                                skip_group_check=True,
                            )
                        if sav and pend is not None:
                            pu, ps = pend
                            pend = None
                            s_mms(pu, ps)
                        fill(nf2)
                        pend = (u, s)
                    else:
                        for j in range(4):
                            nc.tensor.matmul(
                                accP[:],
                                lhsT=sS[:, j * HID : (j + 1) * HID],
                                rhs=u[:, j * BL : (j + 1) * BL],
                                start=False,
                                stop=(j == 3),
                                skip_group_check=True,
                            )
                        fill(nf2)
                znext = zall[:, (step + 1) * BL : (step + 2) * BL]
                nc.vector.scalar_tensor_tensor(
                    out=znext,
                    in0=accP[:],
                    scalar=1.0 / 6.0,
                    in1=zT,
                    op0=mybir.AluOpType.mult,
                    op1=mybir.AluOpType.add,
                )
                if step + 1 < nstep:
                    hT = h_group(zh_copy(znext)[:])
                nc.sync.dma_start(
                    out=zs_out[:, (step + 1) * BL : (step + 2) * BL],
                    in_=znext,
                )

    print(f"[kernel] tile trace+schedule: {_time.time()-t0:.1f}s", file=sys.stderr)
    t1 = _time.time()
    nc.finalize()
    print(f"[kernel] finalize: {_time.time()-t1:.1f}s", file=sys.stderr)
    return nc


def _build2(nstep: int, with_b2: bool):
    """Two-stream pipelined variant: per core, the 128-row batch is split
    into two independent 64-row streams (each its own serial RK4 chain).
    The streams interleave on the engines so the per-stage serial chain
    (relu -> mm2 -> tanh -> mult -> V) of one stream hides under the
    other's work. Engine assignment: relu on DVE (PSUM reader), tanh on
    ACT, dx-mult + u-accumulate + z-cast on Pool, all matmuls on PE.
    No warm fillers: PE stays busy with real work (garbage fillers steal
    clock from the other engines via the shared power budget).

    PSUM (8 banks): per stream 2 h banks (stage pairs (0,2) and (1,3)
    share a bank via the start=True/False has_written pattern), 1 f_ps
    bank, 1 accP bank.
    """
    import sys
    import time as _time

    t0 = _time.time()
    HB = BL // 2  # 64 cols per stream
    nc = bacc.Bacc()
    dx_in = nc.dram_tensor("dxr", [128, nstep * 3 * BL], FP16, kind="ExternalInput")
    b1_in = nc.dram_tensor("bias1", [MLP_H, nstep * 3], F32, kind="ExternalInput")
    w1zh_in = nc.dram_tensor("w1zh", [HID, MLP_H], FP16, kind="ExternalInput")
    w2_in = nc.dram_tensor("w2", [MLP_H, NF], FP16, kind="ExternalInput")
    v_in = nc.dram_tensor("vtab", [128, 2 * NF], FP16, kind="ExternalInput")
    s_in = nc.dram_tensor("spat", [128, 4 * HID], FP16, kind="ExternalInput")
    b2_in = nc.dram_tensor("b2t", [128, 4], F32, kind="ExternalInput")
    z0t_in = nc.dram_tensor("z0t", [HID, BL], F32, kind="ExternalInput")
    zs_out = nc.dram_tensor("zs", [HID, (nstep + 1) * BL], F32, kind="ExternalOutput")
    import os as _os

    if _os.environ.get("K_H0", "1") == "1":
        v6_in = nc.dram_tensor("vt6", [128, NF], FP16, kind="ExternalInput")

    CLS = (0, 1, 1, 2)
    VVAR = (0, 1, 0)
    import os

    dxring = int(os.environ.get("K_DXRING", "8"))
    # engine assignment per stream: first char = stream A, second = B
    # d=DVE, p=Pool(gpsimd), a=ACT
    relu_eng = os.environ.get("K_RELU2", "da")
    mult_eng = os.environ.get("K_MULT", "dd")
    uacc_eng = os.environ.get("K_UACC", "dd")  # or "pe" = S per stage, no uacc
    h0mode = os.environ.get("K_H0", "1") == "1"  # persistent h0 bank + V6 fold
    h0reb = os.environ.get("K_H0REB", "0") == "1"  # debug: rebuild h0 from zh
    nofs = int(os.environ.get("K_OFS", "6"))  # stream-B bootstrap offset chain

    with TileContext(nc) as tc:
        with (
            tc.tile_pool(name="const", bufs=1) as cp,
            tc.tile_pool(name="zst", bufs=1) as zp,
            tc.tile_pool(name="dx", bufs=dxring) as dxp,
            tc.tile_pool(name="hsA", bufs=2) as hpA,
            tc.tile_pool(name="hsB", bufs=2) as hpB,
            tc.tile_pool(name="fsA", bufs=2) as fspA,
            tc.tile_pool(name="fsB", bufs=2) as fspB,
            tc.tile_pool(name="uA", bufs=2) as upA,
            tc.tile_pool(name="uB", bufs=2) as upB,
            tc.tile_pool(name="uaA", bufs=2) as uapA,
            tc.tile_pool(name="uaB", bufs=2) as uapB,
            tc.tile_pool(name="zhA", bufs=2) as zhpA,
            tc.tile_pool(name="zhB", bufs=2) as zhpB,
            tc.tile_pool(name="phA", bufs=1, space="PSUM") as phA,
            tc.tile_pool(name="phB", bufs=1, space="PSUM") as phB,
            tc.tile_pool(name="pfA", bufs=1, space="PSUM") as pfA,
            tc.tile_pool(name="pfB", bufs=1, space="PSUM") as pfB,
            tc.tile_pool(name="paccA", bufs=1, space="PSUM") as paccA,
            tc.tile_pool(name="paccB", bufs=1, space="PSUM") as paccB,
        ):
            b1S = cp.tile([MLP_H, nstep * 3], F32)
            w1zhS = cp.tile([HID, MLP_H], FP16)
            w2S = cp.tile([MLP_H, NF], FP16)
            vS = cp.tile([128, 2 * NF], FP16)
            sS = cp.tile([128, 4 * HID], FP16)
            b2S = cp.tile([128, 4], F32)
            zall = zp.tile([HID, (nstep + 1) * BL], F32)

            nc.sync.dma_start(out=b1S[:], in_=b1_in[:])
            nc.sync.dma_start(out=w1zhS[:], in_=w1zh_in[:])
            nc.sync.dma_start(out=w2S[:], in_=w2_in[:])
            nc.sync.dma_start(out=vS[:], in_=v_in[:])
            nc.sync.dma_start(out=sS[:], in_=s_in[:])
            nc.sync.dma_start(out=b2S[:], in_=b2_in[:])
            nc.sync.dma_start(out=zall[:, 0:BL], in_=z0t_in[:])
            nc.sync.dma_start(out=zs_out[:, 0:BL], in_=z0t_in[:].bitcast(F32))
            if h0mode:
                v6S = cp.tile([128, NF], FP16)
                nc.sync.dma_start(out=v6S[:], in_=v6_in[:])

            pools = [
                dict(hp=hpA, fsp=fspA, up=upA, uap=uapA, zhp=zhpA,
                     ph=phA, pf=pfA, pacc=paccA),
                dict(hp=hpB, fsp=fspB, up=upB, uap=uapB, zhp=zhpB,
                     ph=phB, pf=pfB, pacc=paccB),
            ]

            def zh_cast(X, z_ap):
                t = pools[X]["zhp"].tile([HID, HB], FP16, tag="zh", name=f"zh{X}")
                nc.gpsimd.tensor_copy(t[:], z_ap)
                return t

            def h_group(X, zh_ap):
                """4 h tiles for one stream's next step, packed two per
                bank: bank a holds stages (0, 2), bank b holds (1, 3).
                start=True only on each bank's first write (stages 0, 1);
                stages 2, 3 overwrite-on-clear-bits via start=False."""
                ph = pools[X]["ph"]
                ba = ph.tile([MLP_H, 2 * HB], F32, tag="hba", name=f"hba{X}")
                bb = ph.tile([MLP_H, 2 * HB], F32, tag="hbb", name=f"hbb{X}")
                tiles = [
                    ba[:, 0:HB], bb[:, 0:HB],
                    ba[:, HB : 2 * HB], bb[:, HB : 2 * HB],
                ]
                for s, st in ((0, True), (2, False), (1, True), (3, False)):
                    nc.tensor.matmul(
                        tiles[s],
                        lhsT=w1zhS[:],
                        rhs=zh_ap,
                        start=st,
                        stop=(s == 0),
                        skip_group_check=True,
                    )
                return tiles

            h0t = [None, None]

            T3S = os.environ.get("K_H0T3", "1") == "1"
            HG_START = {1: True, 2: False, 3: T3S}

            def hg_mm(X, zh_ap, tiles, s, st):
                nc.tensor.matmul(
                    tiles[s],
                    lhsT=w1zhS[:],
                    rhs=zh_ap,
                    start=st,
                    stop=False,
                    skip_group_check=True,
                )

            def h_group3(X, zh_ap, boot=False, emit=True):
                """h0mode: tiles 1,2 pack one bank; tile3 shares a bank
                with accP (accP's once-per-step start=True S matmul only
                clears bits after tile3's V accumulation finished, PE
                program order guarantees it). Tile 0 is the persistent
                h0 bank, updated by V6 accumulation, not rebuilt."""
                ph = pools[X]["ph"]
                t12 = ph.tile([MLP_H, 2 * HB], F32, tag="hba", name=f"hba{X}")
                tg = pools[X]["pacc"].tile(
                    [MLP_H, 2 * HB], F32, tag="acc", name=f"acc{X}"
                )
                tiles = [
                    h0t[X][:], t12[:, 0:HB], t12[:, HB : 2 * HB], tg[:, 0:HB],
                ]
                if emit:
                    specs = [(1, True), (2, False), (3, boot or T3S)]
                    if h0reb and not boot:
                        specs = [(0, True)] + specs
                    for s, st in specs:
                        hg_mm(X, zh_ap, tiles, s, st)
                return tiles, tg[0:HID, HB : 2 * HB]

            hT = [None, None]
            accPk = [None, None]
            pend_hg = [None, None]
            pend_tiles = [None, None]

            def bootstrap(X):
                zh0 = zh_cast(X, zall[:, X * HB : (X + 1) * HB])
                if X == 1 and nofs > 0:
                    prev = zh0
                    for _k in range(nofs):
                        t = pools[X]["zhp"].tile(
                            [HID, HB], FP16, tag="zo", name="zo"
                        )
                        nc.vector.tensor_copy(t[:], prev[:])
                        prev = t
                    zh0 = prev
                if h0mode:
                    h0t[X] = pools[X]["ph"].tile(
                        [MLP_H, HB], F32, tag="h0", name=f"h0{X}"
                    )
                    nc.tensor.matmul(
                        h0t[X][:], lhsT=w1zhS[:], rhs=zh0[:],
                        start=True, stop=False, skip_group_check=True,
                    )
                    hT[X], accPk[X] = h_group3(X, zh0[:], boot=True)
                else:
                    hT[X] = h_group(X, zh0[:])

            for X in range(2):
                bootstrap(X)

            st_mult = mybir.AluOpType.mult
            st_add = mybir.AluOpType.add
            st_max = mybir.AluOpType.max

            def eng(spec, X):
                c = spec[X]
                return {"d": nc.vector, "p": nc.gpsimd, "a": nc.scalar}[c]

            use_uacc = uacc_eng != "pe"

            def emit_relu(X, h_ap, col):
                t = pools[X]["hp"].tile([MLP_H, HB], FP16, tag="hs", name=f"hs{X}")
                e = eng(relu_eng, X)
                if e is nc.scalar:
                    nc.scalar.activation(
                        t[:], h_ap, AF.Relu, bias=b1S[:, col : col + 1]
                    )
                else:
                    e.tensor_scalar(
                        t[:], h_ap, b1S[:, col : col + 1], 0.0,
                        op0=st_add, op1=st_max,
                    )
                return t

            def emit_mm2(X, hS):
                t = pools[X]["pf"].tile([128, NF // 2], F32, tag="fps", name=f"fps{X}")
                for j in range(4):
                    nc.tensor.matmul(
                        t[:, j * HB : (j + 1) * HB],
                        lhsT=w2S[:, j * MLP_H : (j + 1) * MLP_H],
                        rhs=hS[:],
                        start=True,
                        stop=True,
                    )
                return t

            def emit_tanh(X, f_ps):
                t = pools[X]["fsp"].tile([128, NF // 2], FP16, tag="fs", name=f"fs{X}")
                if with_b2:
                    for j in range(4):
                        nc.scalar.activation(
                            t[:, j * HB : (j + 1) * HB],
                            f_ps[:, j * HB : (j + 1) * HB],
                            AF.Tanh,
                            bias=b2S[:, j : j + 1],
                        )
                else:
                    nc.scalar.activation(t[:], f_ps[:], AF.Tanh)
                return t

            def emit_mult(X, fS, dxS, cls):
                t = pools[X]["up"].tile([128, NF // 2], FP16, tag="u", name=f"u{X}")
                u3 = t[:].rearrange("p (j b) -> p j b", j=4)
                f3 = fS[:].rearrange("p (j b) -> p j b", j=4)
                dxv = (
                    dxS[:, cls * BL + X * HB : cls * BL + (X + 1) * HB]
                    .unsqueeze(1)
                    .broadcast_to((128, 4, HB))
                )
                eng(mult_eng, X).tensor_tensor(out=u3, in0=f3, in1=dxv, op=st_mult)
                return t

            def emit_v(X, hT_next, u, vo):
                for j in range(4):
                    nc.tensor.matmul(
                        hT_next,
                        lhsT=vS[:, vo + j * MLP_H : vo + (j + 1) * MLP_H],
                        rhs=u[:, j * HB : (j + 1) * HB],
                        start=False,
                        stop=(j == 3),
                        skip_group_check=True,
                    )

            def emit_s(X, acc_ap, rhs_t, start, stop):
                for j in range(4):
                    nc.tensor.matmul(
                        acc_ap,
                        lhsT=sS[:, j * HID : (j + 1) * HID],
                        rhs=rhs_t[:, j * HB : (j + 1) * HB],
                        start=(start and j == 0),
                        stop=(stop and j == 3),
                        skip_group_check=True,
                    )

            def emit_v6(X, rhs_t, stop):
                for j in range(4):
                    nc.tensor.matmul(
                        h0t[X][:],
                        lhsT=v6S[:, j * MLP_H : (j + 1) * MLP_H],
                        rhs=rhs_t[:, j * HB : (j + 1) * HB],
                        start=False,
                        stop=(stop and j == 3),
                        skip_group_check=True,
                    )

            def emit_step(step, hT):
                dxS = dxp.tile([128, 3 * BL], FP16, tag="dx", name="dx")
                nc.sync.dma_start(
                    out=dxS[:], in_=dx_in[:, step * 3 * BL : (step + 1) * 3 * BL]
                )
                accP = [None, None]
                uacc = [None, None]
                u0 = [None, None]
                hS = [None, None]
                f_ps = [None, None]
                fS = [None, None]
                u = [None, None]
                for X in range(2):
                    if h0mode:
                        accP[X] = accPk[X]
                    else:
                        accP[X] = pools[X]["pacc"].tile(
                            [HID, HB], F32, tag="acc", name=f"acc{X}"
                        )[:]
                    if use_uacc:
                        uacc[X] = pools[X]["uap"].tile(
                            [128, NF // 2], FP16, tag="ua", name=f"ua{X}"
                        )
                for s in range(4):
                    col = step * 3 + CLS[s]
                    for X in range(2):
                        hS[X] = emit_relu(X, hT[X][s], col)
                    if s < 3 and pend_hg[0] is not None:
                        # One deferred h-base matmul per stage, emitted
                        # before mm2: it is ready immediately (runs while
                        # relu finishes), absorbing the PE cold-restart
                        # so mm2 runs at full clock. Tile s+1 is only
                        # needed at relu(s+1); its V accumulation (end of
                        # this stage) sits later in the PE queue.
                        for X in range(2):
                            if s == 0:
                                hTn, accPk[X] = h_group3(X, pend_hg[X][:],
                                                         emit=False)
                                pend_tiles[X] = hTn
                                accP[X] = accPk[X]
                            hT[X][s + 1] = pend_tiles[X][s + 1]
                            hg_mm(X, pend_hg[X][:], pend_tiles[X], s + 1,
                                  HG_START[s + 1])
                        if s == 2:
                            pend_hg[0] = pend_hg[1] = None
                    for X in range(2):
                        f_ps[X] = emit_mm2(X, hS[X])
                    if s == 3 and use_uacc:
                        # uacc012 S/V6 matmuls are ready since stage 2;
                        # they drain on PE while tanh3/mult3 run.
                        for X in range(2):
                            emit_s(X, accP[X], uacc[X][:], True, False)
                        if h0mode and not h0reb:
                            for X in range(2):
                                emit_v6(X, uacc[X][:], False)
                    for X in range(2):
                        fS[X] = emit_tanh(X, f_ps[X])
                    for X in range(2):
                        u[X] = emit_mult(X, fS[X], dxS, CLS[s])
                    if use_uacc:
                        # uacc = u0+u1+u2 only; u3 feeds the S matmuls
                        # directly so the final add never sits on the
                        # step-tail chain.
                        for X in range(2):
                            if s == 0:
                                u0[X] = u[X]
                            elif s == 1:
                                eng(uacc_eng, X).tensor_tensor(
                                    out=uacc[X][:], in0=u0[X][:], in1=u[X][:],
                                    op=st_add,
                                )
                            elif s == 2:
                                eng(uacc_eng, X).tensor_tensor(
                                    out=uacc[X][:], in0=uacc[X][:], in1=u[X][:],
                                    op=st_add,
                                )
                    if s < 3:
                        for X in range(2):
                            emit_v(X, hT[X][s + 1], u[X][:], VVAR[s] * NF)
                    if not use_uacc:
                        for X in range(2):
                            emit_s(X, accP[X], u[X][:], s == 0, s == 3)
                # step tail. h0mode: the next stage-0 h comes straight from
                # V6 accumulation on u3 (short chain); the z-space path
                # (S, STT, h tiles 1-3) runs off-chain in parallel.
                if h0mode and not h0reb:
                    for X in range(2):
                        emit_v6(X, u[X][:], True)
                if use_uacc:
                    for X in range(2):
                        emit_s(X, accP[X], u[X][:], False, True)
                newhT = [None, None]
                for X in range(2):
                    zT = zall[:, step * BL + X * HB : step * BL + (X + 1) * HB]
                    znext = zall[
                        :, (step + 1) * BL + X * HB : (step + 1) * BL + (X + 1) * HB
                    ]
                    if step + 1 < nstep:
                        zh = pools[X]["zhp"].tile(
                            [HID, HB], FP16, tag="zh", name=f"zh{X}"
                        )
                        nc.vector.scalar_tensor_tensor(
                            out=zh[:],
                            in0=accP[X],
                            scalar=1.0 / 6.0,
                            in1=zT,
                            op0=st_mult,
                            op1=st_add,
                        )
                        if h0mode and not h0reb:
                            # PE matmuls for tiles 1-3 are deferred into
                            # the next step's stage 0 (post-mm2 slot).
                            newhT[X] = [hT[X][0], None, None, None]
                            pend_hg[X] = zh
                        elif h0mode:
                            newhT[X], accPk[X] = h_group3(X, zh[:])
                        else:
                            newhT[X] = h_group(X, zh[:])
                    nc.vector.scalar_tensor_tensor(
                        out=znext,
                        in0=accP[X],
                        scalar=1.0 / 6.0,
                        in1=zT,
                        op0=st_mult,
                        op1=st_add,
                    )
                    nc.sync.dma_start(
                        out=zs_out[
                            :, (step + 1) * BL + X * HB : (step + 1) * BL + (X + 1) * HB
                        ],
                        in_=znext,
                    )
                return newhT

            for step in range(nstep):
                newhT = emit_step(step, hT)
                if step + 1 < nstep:
                    hT = newhT

    print(f"[kernel2] tile trace+schedule: {_time.time()-t0:.1f}s", file=sys.stderr)
    t1 = _time.time()
    nc.finalize()
    print(f"[kernel2] finalize: {_time.time()-t1:.1f}s", file=sys.stderr)
    return nc


def _use_v2():
    import os

    return os.environ.get("K_V2", "0") == "1"


def _get_nc(nstep: int, with_b2: bool):
    if _use_v2():
        import os as _os

        key = ("v2", nstep, with_b2,
               _os.environ.get("K_RELU2", ""), _os.environ.get("K_MULT", ""),
               _os.environ.get("K_UACC", ""), _os.environ.get("K_H0", ""),
               _os.environ.get("K_OFS", ""), _os.environ.get("K_H0REB", ""))
        if key not in _CACHE:
            _CACHE[key] = _build2(nstep, with_b2)
        return _CACHE[key]
    key = (nstep, with_b2) + _flags()
    if key not in _CACHE:
        _CACHE[key] = _build(nstep, with_b2)
    return _CACHE[key]


def _host_prep(coeffs, Wi1, bi1, Wi2, bi2, W1, b1, W2, b2, nstep: int):
    coeffs = np.asarray(coeffs, dtype=np.float32)
    a = coeffs[:, :, 0:8]
    b = coeffs[:, :, 8:16]
    c = coeffs[:, :, 16:24]
    d = coeffs[:, :, 24:32]

    X0 = a[:, 0]
    z0 = np.tanh(
        np.maximum(X0 @ Wi1 + bi1, 0.0).astype(np.float32) @ Wi2 + bi2
    ).astype(np.float32)

    # dX/dt at the three per-step sample classes, RK4 combine weights
    # folded in: class0 = dX(i) (k1 w=1), class1 = 2*dX(i+0.5) (k2+k3 w=2
    # each), class2 = dX(i+1) (k4 w=1).
    g = np.empty((B, nstep, 3, C_IN), dtype=np.float32)
    g[:, :, 0] = b[:, :nstep]
    g[:, :, 1] = 2.0 * b[:, :nstep] + 2.0 * c[:, :nstep] + 1.5 * d[:, :nstep]
    last = NSTEP - 1
    for i in range(nstep):
        if i < last:
            g[:, i, 2] = b[:, i + 1]
        else:
            g[:, i, 2] = b[:, i] + 2.0 * c[:, i] + 3.0 * d[:, i]

    tcols = np.empty((nstep, 3), dtype=np.float32)
    tcols[:, 0] = np.arange(nstep, dtype=np.float32)
    tcols[:, 1] = tcols[:, 0] + 0.5
    tcols[:, 2] = tcols[:, 0] + 1.0
    bias1 = (
        b1[None, None, :] + tcols[:, :, None] * W1[0][None, None, :]
    ).astype(np.float32)
    bias1 = bias1.reshape(nstep * 3, MLP_H).T.copy()

    w1z = np.ascontiguousarray(W1[1:], dtype=np.float32)  # (64, 128)
    # V tables: (S_j @ W1z) * alpha, laid out [p, (variant, j, m)]
    vfull = np.repeat(W1[1:], C_IN, axis=0).astype(np.float32)  # (512, 128)
    vt = np.stack([0.5 * vfull, 0.25 * vfull])  # (2, 512, 128)
    vt = vt.reshape(2, 4, 128, MLP_H).transpose(2, 0, 1, 3)
    vtab = np.ascontiguousarray(vt.reshape(128, 2 * NF), dtype=np.float16)
    # S pattern: group-of-8 partition sum, [p, (j, m)]
    q = np.arange(NF)
    sfull = (q[:, None] // C_IN == np.arange(HID)[None, :]).astype(np.float16)
    spat = np.ascontiguousarray(
        sfull.reshape(4, 128, HID).transpose(1, 0, 2).reshape(128, 4 * HID)
    )
    # b2 per-partition chunks [p, j] (only used when b2 != 0)
    b2t = np.ascontiguousarray(
        np.asarray(b2, np.float32).reshape(4, 128).T
    )

    v6 = (vfull / 6.0).reshape(4, 128, MLP_H).transpose(1, 0, 2)
    vt6 = np.ascontiguousarray(v6.reshape(128, NF), dtype=np.float16)

    shared = {
        "bias1": bias1,
        "w1z": w1z,
        "w1zh": np.ascontiguousarray(W1[1:], dtype=np.float16),
        "w2": np.ascontiguousarray(W2, dtype=np.float16),
        "vtab": vtab,
        "spat": spat,
        "b2t": b2t,
        "vt6": vt6,
    }
    in_maps = []
    for core in range(NCORES):
        sl = slice(core * BL, (core + 1) * BL)
        m = dict(shared)
        arr = g[sl].astype(np.float16)  # (BL, nstep, 3, 8)
        arr = arr.transpose(3, 1, 2, 0)  # (8, nstep, 3, BL)
        arr = np.tile(arr, (16, 1, 1, 1))  # (128, nstep, 3, BL); p%8 = c
        if _flags()[4]:
            arr = np.repeat(arr[:, :, :, None, :], 4, axis=3)
            m["dxr"] = np.ascontiguousarray(arr.reshape(128, nstep * 3 * 4 * BL))
        else:
            m["dxr"] = np.ascontiguousarray(arr.reshape(128, nstep * 3 * BL))
        m["z0t"] = np.ascontiguousarray(z0[sl].T)
        in_maps.append(m)
    return in_maps, z0


def kernel(coeffs, Wi1, bi1, Wi2, bi2, W1, b1, W2, b2, _nstep: int = NSTEP,
           _trace: bool = False):
    import sys
    import time as _time

    nstep = _nstep
    with_b2 = bool(np.any(np.asarray(b2)))
    nc = _get_nc(nstep, with_b2)
    in_maps, _ = _host_prep(
        coeffs, Wi1, bi1, Wi2, bi2, W1, b1, W2, b2, nstep
    )
    import os as _os

    v3 = _use_v2() and _os.environ.get("K_H0", "1") == "1"
    for m in in_maps:
        if _use_v2():
            m.pop("w1z", None)
        if not v3:
            m.pop("vt6", None)
    t0 = _time.time()
    res = run_bass_kernel_spmd(nc, in_maps, list(range(NCORES)), trace=_trace)
    print(f"[kernel] spmd run (compile+exec): {_time.time()-t0:.1f}s", file=sys.stderr)
    out = np.empty((B, nstep + 1, HID), dtype=np.float32)
    for core in range(NCORES):
        zs = res.results[core]["zs"].reshape(HID, nstep + 1, BL)
        out[core * BL : (core + 1) * BL] = zs.transpose(2, 1, 0)
    if _trace:
        kernel.last_results = res
    return out



# revision 47
# speedup vs baseline: 1.0936x; 1.0936x over previous
"""NeuralCDE RK4 solver as a Bass/Tile kernel on 8 Trainium2 cores.

Data-parallel over batch: B=1024 -> 128 rows per core. The 127-step RK4
scan is fully unrolled. Everything lives in T layout (features on
partitions, batch on the free dim); per stage the critical chain is:

    relu (ACT): hS[128m,128b] = relu(h_psum + bias1(t))      (t folded in bias)
    mm2T (PE) : f_ps[128hc,4*128b] = W2chunk_j.T @ hS        (4 MMs, one bank)
    tanh (ACT): fS = tanh(f_ps)                              (b2 == 0)
    mul  (DVE): u = fS * dxrep(step,cls)                     (dx broadcast over j)
    V    (PE) : h_psum[s+1] += (S_j@W1z*alpha).T @ u_j       (4 MMs; k never
                materialized -- feeds next stage's mm1 directly)
    S    (PE) : accP[64,128b] += S_j.T @ u_j                 (RK4 sum in PSUM)

Off the chain: one z-part matmul per stage (W1z.T @ z), one DVE op per
step for z_{i+1} = z + accP/6, and streamed DMAs for the host-built
dxrep tables (dX/dt values pre-replicated over the 16 h-groups, RK4
stage weights folded in).
"""

import numpy as np

import concourse.bacc as bacc
import concourse.bass as bass
import concourse.mybir as mybir
from concourse.tile import TileContext
from concourse.bass_utils import run_bass_kernel_spmd

F32 = mybir.dt.float32
FP16 = mybir.dt.float16
AF = mybir.ActivationFunctionType

B = 1024
L = 128
C_IN = 8
HID = 64
MLP_H = 128
NSTEP = L - 1  # 127
NCORES = 8
BL = B // NCORES  # 128 batch rows per core
NF = HID * C_IN  # 512

_CACHE: dict = {}


def _flags():
    import os

    return (
        os.environ.get("K_WARM", "1") == "1",
        int(os.environ.get("K_DXRING", "16")),
        int(os.environ.get("K_NFILL", "12")),
        os.environ.get("K_RELU_DVE", "0") == "1",
        os.environ.get("K_DX4", "0") == "1",
        int(os.environ.get("K_NF2", "0")),  # fillers after V (post-V window)
        int(os.environ.get("K_FW", "256")),  # filler free width
        os.environ.get("K_SAV", "0") == "1",  # pend-S after V
    )


def _build(nstep: int, with_b2: bool):
    import sys
    import time as _time

    warm, dxring, nfill, relu_dve, dx4, nf2, fw, sav = _flags()
    t0 = _time.time()
    nc = bacc.Bacc()
    DXW = 4 * BL if dx4 else BL
    dx_in = nc.dram_tensor("dxr", [128, nstep * 3 * DXW], FP16, kind="ExternalInput")
    b1_in = nc.dram_tensor("bias1", [MLP_H, nstep * 3], F32, kind="ExternalInput")
    w1z_in = nc.dram_tensor("w1z", [HID, MLP_H], F32, kind="ExternalInput")
    w1zh_in = nc.dram_tensor("w1zh", [HID, MLP_H], FP16, kind="ExternalInput")
    w2_in = nc.dram_tensor("w2", [MLP_H, NF], FP16, kind="ExternalInput")
    v_in = nc.dram_tensor("vtab", [128, 2 * NF], FP16, kind="ExternalInput")
    s_in = nc.dram_tensor("spat", [128, 4 * HID], FP16, kind="ExternalInput")
    b2_in = nc.dram_tensor("b2t", [128, 4], F32, kind="ExternalInput")
    z0t_in = nc.dram_tensor("z0t", [HID, BL], F32, kind="ExternalInput")
    zs_out = nc.dram_tensor(
        "zs", [HID, (nstep + 1) * BL], F32, kind="ExternalOutput"
    )

    CLS = (0, 1, 1, 2)
    # V-table variant per stage transition s -> s+1 (alpha 0.5, 0.25, 0.5)
    VVAR = (0, 1, 0)

    with TileContext(nc) as tc:
        with (
            tc.tile_pool(name="const", bufs=1) as cp,
            tc.tile_pool(name="zst", bufs=1) as zp,
            tc.tile_pool(name="dx", bufs=dxring) as dxp,
            tc.tile_pool(name="hs", bufs=3) as hp,
            tc.tile_pool(name="fs", bufs=2) as fsp,
            tc.tile_pool(name="us", bufs=3) as up,
            tc.tile_pool(name="zh", bufs=2) as zhp,
            tc.tile_pool(name="ph", bufs=4, space="PSUM") as ph,
            tc.tile_pool(name="pf", bufs=2, space="PSUM") as pf,
            tc.tile_pool(name="pacc", bufs=1, space="PSUM") as pacc,
            tc.tile_pool(name="pfill", bufs=1, space="PSUM") as pfill,
        ):
            b1S = cp.tile([MLP_H, nstep * 3], F32)
            w1zS = cp.tile([HID, MLP_H], F32)
            w1zhS = cp.tile([HID, MLP_H], FP16)
            w2S = cp.tile([MLP_H, NF], FP16)
            vS = cp.tile([128, 2 * NF], FP16)
            sS = cp.tile([128, 4 * HID], FP16)
            b2S = cp.tile([128, 4], F32)
            zall = zp.tile([HID, (nstep + 1) * BL], F32)

            nc.sync.dma_start(out=b1S[:], in_=b1_in[:])
            nc.sync.dma_start(out=w1zS[:], in_=w1z_in[:])
            nc.sync.dma_start(out=w1zhS[:], in_=w1zh_in[:])
            nc.sync.dma_start(out=w2S[:], in_=w2_in[:])
            nc.sync.dma_start(out=vS[:], in_=v_in[:])
            nc.sync.dma_start(out=sS[:], in_=s_in[:])
            nc.sync.dma_start(out=b2S[:], in_=b2_in[:])
            nc.sync.dma_start(out=zall[:, 0:BL], in_=z0t_in[:])
            nc.sync.dma_start(out=zs_out[:, 0:BL], in_=z0t_in[:].bitcast(F32))

            if warm:
                wtL = cp.tile([128, MLP_H], FP16, name="wtL")
                wtR = cp.tile([128, fw], FP16, name="wtR")
                nc.vector.memset(wtL[:], 0.0)
                nc.vector.memset(wtR[:], 0.0)
                wp = pfill.tile([128, NF], F32, tag="fl", name="wp")

            def fill(n):
                if warm:
                    for _f in range(n):
                        nc.tensor.matmul(
                            wp[:, 0:fw], lhsT=wtL[:], rhs=wtR[:],
                            start=True, stop=True,
                        )

            def zh_copy(z_ap):
                t = zhp.tile([HID, BL], FP16, tag="zh", name="zh")
                nc.vector.tensor_copy(t[:], z_ap)
                return t

            def h_group(znext_ap):
                """Open a step's 4 h PSUM tiles (one bank each -- start=True
                clears has_written bank-wide, so stages can't share a bank):
                z-part matmuls. Tile 0 (stage 0) is complete on its own;
                tiles 1..3 get V-matmul accumulation during stages 0..2.
                """
                tiles = []
                for s in range(4):
                    t = ph.tile([MLP_H, BL], F32, tag="hps", name=f"hps{s}")
                    nc.tensor.matmul(
                        t[:],
                        lhsT=w1zhS[:],
                        rhs=znext_ap,
                        start=True,
                        stop=(s == 0),
                        skip_group_check=True,
                    )
                    tiles.append(t)
                return tiles

            hT = h_group(zh_copy(zall[:, 0:BL])[:])

            for step in range(nstep):
                dxS = dxp.tile([128, 3 * DXW], FP16, tag="dx", name="dx")
                nc.sync.dma_start(
                    out=dxS[:], in_=dx_in[:, step * 3 * DXW : (step + 1) * 3 * DXW]
                )
                zT = zall[:, step * BL : (step + 1) * BL]
                accP = pacc.tile([HID, BL], F32, tag="acc", name="acc")
                pend = None
                for s in range(4):
                    col = step * 3 + CLS[s]
                    hS = hp.tile([MLP_H, BL], FP16, tag="hs", name="hs")
                    if relu_dve:
                        nc.vector.tensor_scalar(
                            hS[:],
                            hT[s][:],
                            b1S[:, col : col + 1],
                            0.0,
                            op0=mybir.AluOpType.add,
                            op1=mybir.AluOpType.max,
                        )
                    else:
                        nc.scalar.activation(
                            hS[:],
                            hT[s][:],
                            AF.Relu,
                            bias=b1S[:, col : col + 1],
                        )
                    f_ps = pf.tile([128, NF], F32, tag="fps", name="fps")
                    for j in range(4):
                        nc.tensor.matmul(
                            f_ps[:, j * BL : (j + 1) * BL],
                            lhsT=w2S[:, j * MLP_H : (j + 1) * MLP_H],
                            rhs=hS[:],
                            start=True,
                            stop=True,
                        )

                    def s_mms(pu, ps):
                        for j in range(4):
                            nc.tensor.matmul(
                                accP[:],
                                lhsT=sS[:, j * HID : (j + 1) * HID],
                                rhs=pu[:, j * BL : (j + 1) * BL],
                                start=(ps == 0 and j == 0),
                                stop=False,
                                skip_group_check=True,
                            )

                    if pend is not None and (not sav or s == 3):
                        pu, ps = pend
                        pend = None
                        s_mms(pu, ps)
                    fill(nfill)
                    fS = fsp.tile([128, NF], FP16, tag="fs", name="fs")
                    if with_b2:
                        for j in range(4):
                            nc.scalar.activation(
                                fS[:, j * BL : (j + 1) * BL],
                                f_ps[:, j * BL : (j + 1) * BL],
                                AF.Tanh,
                                bias=b2S[:, j : j + 1],
                            )
                    else:
                        nc.scalar.activation(fS[:], f_ps[:], AF.Tanh)
                    u = up.tile([128, NF], FP16, tag="u", name="u")
                    if dx4:
                        nc.vector.tensor_tensor(
                            out=u[:],
                            in0=fS[:],
                            in1=dxS[:, CLS[s] * NF : (CLS[s] + 1) * NF],
                            op=mybir.AluOpType.mult,
                        )
                    else:
                        u3 = u[:].rearrange("p (j b) -> p j b", j=4)
                        f3 = fS[:].rearrange("p (j b) -> p j b", j=4)
                        dxv = (
                            dxS[:, CLS[s] * BL : (CLS[s] + 1) * BL]
                            .unsqueeze(1)
                            .broadcast_to((128, 4, BL))
                        )
                        nc.vector.tensor_tensor(
                            out=u3, in0=f3, in1=dxv, op=mybir.AluOpType.mult
                        )
                    if s < 3:
                        voff = VVAR[s] * NF
                        for j in range(4):
                            nc.tensor.matmul(
                                hT[s + 1][:],
                                lhsT=vS[:, voff + j * MLP_H : voff + (j + 1) * MLP_H],
                                rhs=u[:, j * BL : (j + 1) * BL],
                                start=False,
                                stop=(j == 3),
                                skip_group_check=True,
                            )
                        if sav and pend is not None:
                            pu, ps = pend
                            pend = None
                            s_mms(pu, ps)
                        fill(nf2)
                        pend = (u, s)
                    else:
                        for j in range(4):
                            nc.tensor.matmul(
                                accP[:],
                                lhsT=sS[:, j * HID : (j + 1) * HID],
                                rhs=u[:, j * BL : (j + 1) * BL],
                                start=False,
                                stop=(j == 3),
                                skip_group_check=True,
                            )
                        fill(nf2)
                znext = zall[:, (step + 1) * BL : (step + 2) * BL]
                nc.vector.scalar_tensor_tensor(
                    out=znext,
                    in0=accP[:],
                    scalar=1.0 / 6.0,
                    in1=zT,
                    op0=mybir.AluOpType.mult,
                    op1=mybir.AluOpType.add,
                )
                if step + 1 < nstep:
                    hT = h_group(zh_copy(znext)[:])
                nc.sync.dma_start(
                    out=zs_out[:, (step + 1) * BL : (step + 2) * BL],
                    in_=znext,
                )

    print(f"[kernel] tile trace+schedule: {_time.time()-t0:.1f}s", file=sys.stderr)
    t1 = _time.time()
    nc.finalize()
    print(f"[kernel] finalize: {_time.time()-t1:.1f}s", file=sys.stderr)
    return nc


def _build2(nstep: int, with_b2: bool):
    """Two-stream pipelined variant: per core, the 128-row batch is split
    into two independent 64-row streams (each its own serial RK4 chain).
    The streams interleave on the engines so the per-stage serial chain
    (relu -> mm2 -> tanh -> mult -> V) of one stream hides under the
    other's work. Engine assignment: relu on DVE (PSUM reader), tanh on
    ACT, dx-mult + u-accumulate + z-cast on Pool, all matmuls on PE.
    No warm fillers: PE stays busy with real work (garbage fillers steal
    clock from the other engines via the shared power budget).

    PSUM (8 banks): per stream 2 h banks (stage pairs (0,2) and (1,3)
    share a bank via the start=True/False has_written pattern), 1 f_ps
    bank, 1 accP bank.
    """
    import sys
    import time as _time

    t0 = _time.time()
    HB = BL // 2  # 64 cols per stream
    nc = bacc.Bacc()
    dx_in = nc.dram_tensor("dxr", [128, nstep * 3 * BL], FP16, kind="ExternalInput")
    b1_in = nc.dram_tensor("bias1", [MLP_H, nstep * 3], F32, kind="ExternalInput")
    w1zh_in = nc.dram_tensor("w1zh", [HID, MLP_H], FP16, kind="ExternalInput")
    w2_in = nc.dram_tensor("w2", [MLP_H, NF], FP16, kind="ExternalInput")
    v_in = nc.dram_tensor("vtab", [128, 2 * NF], FP16, kind="ExternalInput")
    s_in = nc.dram_tensor("spat", [128, 4 * HID], FP16, kind="ExternalInput")
    b2_in = nc.dram_tensor("b2t", [128, 4], F32, kind="ExternalInput")
    z0t_in = nc.dram_tensor("z0t", [HID, BL], F32, kind="ExternalInput")
    zs_out = nc.dram_tensor("zs", [HID, (nstep + 1) * BL], F32, kind="ExternalOutput")
    import os as _os

    if _os.environ.get("K_H0", "1") == "1":
        v6_in = nc.dram_tensor("vt6", [128, NF], FP16, kind="ExternalInput")

    CLS = (0, 1, 1, 2)
    VVAR = (0, 1, 0)
    import os

    dxring = int(os.environ.get("K_DXRING", "8"))
    # engine assignment per stream: first char = stream A, second = B
    # d=DVE, p=Pool(gpsimd), a=ACT
    relu_eng = os.environ.get("K_RELU2", "da")
    mult_eng = os.environ.get("K_MULT", "dd")
    uacc_eng = os.environ.get("K_UACC", "dd")  # or "pe" = S per stage, no uacc
    h0mode = os.environ.get("K_H0", "1") == "1"  # persistent h0 bank + V6 fold
    h0reb = os.environ.get("K_H0REB", "0") == "1"  # debug: rebuild h0 from zh
    nofs = int(os.environ.get("K_OFS", "6"))  # stream-B bootstrap offset chain

    with TileContext(nc) as tc:
        with (
            tc.tile_pool(name="const", bufs=1) as cp,
            tc.tile_pool(name="zst", bufs=1) as zp,
            tc.tile_pool(name="dx", bufs=dxring) as dxp,
            tc.tile_pool(name="hsA", bufs=2) as hpA,
            tc.tile_pool(name="hsB", bufs=2) as hpB,
            tc.tile_pool(name="fsA", bufs=2) as fspA,
            tc.tile_pool(name="fsB", bufs=2) as fspB,
            tc.tile_pool(name="uA", bufs=2) as upA,
            tc.tile_pool(name="uB", bufs=2) as upB,
            tc.tile_pool(name="uaA", bufs=2) as uapA,
            tc.tile_pool(name="uaB", bufs=2) as uapB,
            tc.tile_pool(name="zhA", bufs=2) as zhpA,
            tc.tile_pool(name="zhB", bufs=2) as zhpB,
            tc.tile_pool(name="phA", bufs=1, space="PSUM") as phA,
            tc.tile_pool(name="phB", bufs=1, space="PSUM") as phB,
            tc.tile_pool(name="pfA", bufs=1, space="PSUM") as pfA,
            tc.tile_pool(name="pfB", bufs=1, space="PSUM") as pfB,
            tc.tile_pool(name="paccA", bufs=1, space="PSUM") as paccA,
            tc.tile_pool(name="paccB", bufs=1, space="PSUM") as paccB,
        ):
            b1S = cp.tile([MLP_H, nstep * 3], F32)
            w1zhS = cp.tile([HID, MLP_H], FP16)
            w2S = cp.tile([MLP_H, NF], FP16)
            vS = cp.tile([128, 2 * NF], FP16)
            sS = cp.tile([128, 4 * HID], FP16)
            b2S = cp.tile([128, 4], F32)
            zall = zp.tile([HID, (nstep + 1) * BL], F32)

            nc.sync.dma_start(out=b1S[:], in_=b1_in[:])
            nc.sync.dma_start(out=w1zhS[:], in_=w1zh_in[:])
            nc.sync.dma_start(out=w2S[:], in_=w2_in[:])
            nc.sync.dma_start(out=vS[:], in_=v_in[:])
            nc.sync.dma_start(out=sS[:], in_=s_in[:])
            nc.sync.dma_start(out=b2S[:], in_=b2_in[:])
            nc.sync.dma_start(out=zall[:, 0:BL], in_=z0t_in[:])
            nc.sync.dma_start(out=zs_out[:, 0:BL], in_=z0t_in[:].bitcast(F32))
            if h0mode:
                v6S = cp.tile([128, NF], FP16)
                nc.sync.dma_start(out=v6S[:], in_=v6_in[:])

            pools = [
                dict(hp=hpA, fsp=fspA, up=upA, uap=uapA, zhp=zhpA,
                     ph=phA, pf=pfA, pacc=paccA),
                dict(hp=hpB, fsp=fspB, up=upB, uap=uapB, zhp=zhpB,
                     ph=phB, pf=pfB, pacc=paccB),
            ]

            def zh_cast(X, z_ap):
                t = pools[X]["zhp"].tile([HID, HB], FP16, tag="zh", name=f"zh{X}")
                nc.gpsimd.tensor_copy(t[:], z_ap)
                return t

            def h_group(X, zh_ap):
                """4 h tiles for one stream's next step, packed two per
                bank: bank a holds stages (0, 2), bank b holds (1, 3).
                start=True only on each bank's first write (stages 0, 1);
                stages 2, 3 overwrite-on-clear-bits via start=False."""
                ph = pools[X]["ph"]
                ba = ph.tile([MLP_H, 2 * HB], F32, tag="hba", name=f"hba{X}")
                bb = ph.tile([MLP_H, 2 * HB], F32, tag="hbb", name=f"hbb{X}")
                tiles = [
                    ba[:, 0:HB], bb[:, 0:HB],
                    ba[:, HB : 2 * HB], bb[:, HB : 2 * HB],
                ]
                for s, st in ((0, True), (2, False), (1, True), (3, False)):
                    nc.tensor.matmul(
                        tiles[s],
                        lhsT=w1zhS[:],
                        rhs=zh_ap,
                        start=st,
                        stop=(s == 0),
                        skip_group_check=True,
                    )
                return tiles

            h0t = [None, None]

            T3S = os.environ.get("K_H0T3", "1") == "1"
            HG_START = {1: True, 2: False, 3: T3S}

            def hg_mm(X, zh_ap, tiles, s, st):
                nc.tensor.matmul(
                    tiles[s],
                    lhsT=w1zhS[:],
                    rhs=zh_ap,
                    start=st,
                    stop=False,
                    skip_group_check=True,
                )

            def h_group3(X, zh_ap, boot=False, emit=True):
                """h0mode: tiles 1,2 pack one bank; tile3 shares a bank
                with accP (accP's once-per-step start=True S matmul only
                clears bits after tile3's V accumulation finished, PE
                program order guarantees it). Tile 0 is the persistent
                h0 bank, updated by V6 accumulation, not rebuilt."""
                ph = pools[X]["ph"]
                t12 = ph.tile([MLP_H, 2 * HB], F32, tag="hba", name=f"hba{X}")
                tg = pools[X]["pacc"].tile(
                    [MLP_H, 2 * HB], F32, tag="acc", name=f"acc{X}"
                )
                tiles = [
                    h0t[X][:], t12[:, 0:HB], t12[:, HB : 2 * HB], tg[:, 0:HB],
                ]
                if emit:
                    specs = [(1, True), (2, False), (3, boot or T3S)]
                    if h0reb and not boot:
                        specs = [(0, True)] + specs
                    for s, st in specs:
                        hg_mm(X, zh_ap, tiles, s, st)
                return tiles, tg[0:HID, HB : 2 * HB]

            hT = [None, None]
            accPk = [None, None]
            pend_hg = [None, None]
            pend_tiles = [None, None]

            def bootstrap(X):
                zh0 = zh_cast(X, zall[:, X * HB : (X + 1) * HB])
                if X == 1 and nofs > 0:
                    prev = zh0
                    for _k in range(nofs):
                        t = pools[X]["zhp"].tile(
                            [HID, HB], FP16, tag="zo", name="zo"
                        )
                        nc.vector.tensor_copy(t[:], prev[:])
                        prev = t
                    zh0 = prev
                if h0mode:
                    h0t[X] = pools[X]["ph"].tile(
                        [MLP_H, HB], F32, tag="h0", name=f"h0{X}"
                    )
                    nc.tensor.matmul(
                        h0t[X][:], lhsT=w1zhS[:], rhs=zh0[:],
                        start=True, stop=False, skip_group_check=True,
                    )
                    hT[X], accPk[X] = h_group3(X, zh0[:], boot=True)
                else:
                    hT[X] = h_group(X, zh0[:])

            for X in range(2):
                bootstrap(X)

            st_mult = mybir.AluOpType.mult
            st_add = mybir.AluOpType.add
            st_max = mybir.AluOpType.max

            def eng(spec, X):
                c = spec[X]
                return {"d": nc.vector, "p": nc.gpsimd, "a": nc.scalar}[c]

            use_uacc = uacc_eng != "pe"

            def emit_relu(X, h_ap, col):
                t = pools[X]["hp"].tile([MLP_H, HB], FP16, tag="hs", name=f"hs{X}")
                e = eng(relu_eng, X)
                if e is nc.scalar:
                    nc.scalar.activation(
                        t[:], h_ap, AF.Relu, bias=b1S[:, col : col + 1]
                    )
                else:
                    e.tensor_scalar(
                        t[:], h_ap, b1S[:, col : col + 1], 0.0,
                        op0=st_add, op1=st_max,
                    )
                return t

            def emit_mm2(X, hS):
                t = pools[X]["pf"].tile([128, NF // 2], F32, tag="fps", name=f"fps{X}")
                for j in range(4):
                    nc.tensor.matmul(
                        t[:, j * HB : (j + 1) * HB],
                        lhsT=w2S[:, j * MLP_H : (j + 1) * MLP_H],
                        rhs=hS[:],
                        start=True,
                        stop=True,
                    )
                return t

            def emit_tanh(X, f_ps):
                t = pools[X]["fsp"].tile([128, NF // 2], FP16, tag="fs", name=f"fs{X}")
                if with_b2:
                    for j in range(4):
                        nc.scalar.activation(
                            t[:, j * HB : (j + 1) * HB],
                            f_ps[:, j * HB : (j + 1) * HB],
                            AF.Tanh,
                            bias=b2S[:, j : j + 1],
                        )
                else:
                    nc.scalar.activation(t[:], f_ps[:], AF.Tanh)
                return t

            def emit_mult(X, fS, dxS, cls):
                t = pools[X]["up"].tile([128, NF // 2], FP16, tag="u", name=f"u{X}")
                u3 = t[:].rearrange("p (j b) -> p j b", j=4)
                f3 = fS[:].rearrange("p (j b) -> p j b", j=4)
                dxv = (
                    dxS[:, cls * BL + X * HB : cls * BL + (X + 1) * HB]
                    .unsqueeze(1)
                    .broadcast_to((128, 4, HB))
                )
                eng(mult_eng, X).tensor_tensor(out=u3, in0=f3, in1=dxv, op=st_mult)
                return t

            def emit_v(X, hT_next, u, vo):
                for j in range(4):
                    nc.tensor.matmul(
                        hT_next,
                        lhsT=vS[:, vo + j * MLP_H : vo + (j + 1) * MLP_H],
                        rhs=u[:, j * HB : (j + 1) * HB],
                        start=False,
                        stop=(j == 3),
                        skip_group_check=True,
                    )

            def emit_s(X, acc_ap, rhs_t, start, stop):
                for j in range(4):
                    nc.tensor.matmul(
                        acc_ap,
                        lhsT=sS[:, j * HID : (j + 1) * HID],
                        rhs=rhs_t[:, j * HB : (j + 1) * HB],
                        start=(start and j == 0),
                        stop=(stop and j == 3),
                        skip_group_check=True,
                    )

            def emit_v6(X, rhs_t, stop):
                for j in range(4):
                    nc.tensor.matmul(
                        h0t[X][:],
                        lhsT=v6S[:, j * MLP_H : (j + 1) * MLP_H],
                        rhs=rhs_t[:, j * HB : (j + 1) * HB],
                        start=False,
                        stop=(stop and j == 3),
                        skip_group_check=True,
                    )

            def emit_step(step, hT):
                dxS = dxp.tile([128, 3 * BL], FP16, tag="dx", name="dx")
                nc.sync.dma_start(
                    out=dxS[:], in_=dx_in[:, step * 3 * BL : (step + 1) * 3 * BL]
                )
                accP = [None, None]
                uacc = [None, None]
                u0 = [None, None]
                hS = [None, None]
                f_ps = [None, None]
                fS = [None, None]
                u = [None, None]
                for X in range(2):
                    if h0mode:
                        accP[X] = accPk[X]
                    else:
                        accP[X] = pools[X]["pacc"].tile(
                            [HID, HB], F32, tag="acc", name=f"acc{X}"
                        )[:]
                    if use_uacc:
                        uacc[X] = pools[X]["uap"].tile(
                            [128, NF // 2], FP16, tag="ua", name=f"ua{X}"
                        )
                for s in range(4):
                    col = step * 3 + CLS[s]
                    for X in range(2):
                        hS[X] = emit_relu(X, hT[X][s], col)
                    for X in range(2):
                        f_ps[X] = emit_mm2(X, hS[X])
                    if s == 0 and pend_hg[0] is not None:
                        # deferred h-group matmuls for tiles 1-3 of this
                        # step: they drain on PE while tanh(s0) runs and
                        # are only needed at relu(s1).
                        for X in range(2):
                            hTn, accPk[X] = h_group3(X, pend_hg[X][:])
                            hT[X][1] = hTn[1]
                            hT[X][2] = hTn[2]
                            hT[X][3] = hTn[3]
                            accP[X] = accPk[X]
                        pend_hg[0] = pend_hg[1] = None
                    if s == 3 and use_uacc:
                        # uacc012 S/V6 matmuls are ready since stage 2;
                        # they drain on PE while tanh3/mult3 run.
                        for X in range(2):
                            emit_s(X, accP[X], uacc[X][:], True, False)
                        if h0mode and not h0reb:
                            for X in range(2):
                                emit_v6(X, uacc[X][:], False)
                    for X in range(2):
                        fS[X] = emit_tanh(X, f_ps[X])
                    for X in range(2):
                        u[X] = emit_mult(X, fS[X], dxS, CLS[s])
                    if use_uacc:
                        # uacc = u0+u1+u2 only; u3 feeds the S matmuls
                        # directly so the final add never sits on the
                        # step-tail chain.
                        for X in range(2):
                            if s == 0:
                                u0[X] = u[X]
                            elif s == 1:
                                eng(uacc_eng, X).tensor_tensor(
                                    out=uacc[X][:], in0=u0[X][:], in1=u[X][:],
                                    op=st_add,
                                )
                            elif s == 2:
                                eng(uacc_eng, X).tensor_tensor(
                                    out=uacc[X][:], in0=uacc[X][:], in1=u[X][:],
                                    op=st_add,
                                )
                    if s < 3:
                        for X in range(2):
                            emit_v(X, hT[X][s + 1], u[X][:], VVAR[s] * NF)
                    if not use_uacc:
                        for X in range(2):
                            emit_s(X, accP[X], u[X][:], s == 0, s == 3)
                # step tail. h0mode: the next stage-0 h comes straight from
                # V6 accumulation on u3 (short chain); the z-space path
                # (S, STT, h tiles 1-3) runs off-chain in parallel.
                if h0mode and not h0reb:
                    for X in range(2):
                        emit_v6(X, u[X][:], True)
                if use_uacc:
                    for X in range(2):
                        emit_s(X, accP[X], u[X][:], False, True)
                newhT = [None, None]
                for X in range(2):
                    zT = zall[:, step * BL + X * HB : step * BL + (X + 1) * HB]
                    znext = zall[
                        :, (step + 1) * BL + X * HB : (step + 1) * BL + (X + 1) * HB
                    ]
                    if step + 1 < nstep:
                        zh = pools[X]["zhp"].tile(
                            [HID, HB], FP16, tag="zh", name=f"zh{X}"
                        )
                        nc.vector.scalar_tensor_tensor(
                            out=zh[:],
                            in0=accP[X],
                            scalar=1.0 / 6.0,
                            in1=zT,
                            op0=st_mult,
                            op1=st_add,
                        )
                        if h0mode and not h0reb:
                            # PE matmuls for tiles 1-3 are deferred into
                            # the next step's stage 0 (post-mm2 slot).
                            newhT[X] = [hT[X][0], None, None, None]
                            pend_hg[X] = zh
                        elif h0mode:
                            newhT[X], accPk[X] = h_group3(X, zh[:])
                        else:
                            newhT[X] = h_group(X, zh[:])
                    nc.vector.scalar_tensor_tensor(
                        out=znext,
                        in0=accP[X],
                        scalar=1.0 / 6.0,
                        in1=zT,
                        op0=st_mult,
                        op1=st_add,
                    )
                    nc.sync.dma_start(
                        out=zs_out[
                            :, (step + 1) * BL + X * HB : (step + 1) * BL + (X + 1) * HB
                        ],
                        in_=znext,
                    )
                return newhT

            for step in range(nstep):
                newhT = emit_step(step, hT)
                if step + 1 < nstep:
                    hT = newhT

    print(f"[kernel2] tile trace+schedule: {_time.time()-t0:.1f}s", file=sys.stderr)
    t1 = _time.time()
    nc.finalize()
    print(f"[kernel2] finalize: {_time.time()-t1:.1f}s", file=sys.stderr)
    return nc


def _use_v2():
    import os

    return os.environ.get("K_V2", "0") == "1"


def _get_nc(nstep: int, with_b2: bool):
    if _use_v2():
        import os as _os

        key = ("v2", nstep, with_b2,
               _os.environ.get("K_RELU2", ""), _os.environ.get("K_MULT", ""),
               _os.environ.get("K_UACC", ""), _os.environ.get("K_H0", ""),
               _os.environ.get("K_OFS", ""), _os.environ.get("K_H0REB", ""))
        if key not in _CACHE:
            _CACHE[key] = _build2(nstep, with_b2)
        return _CACHE[key]
    key = (nstep, with_b2) + _flags()
    if key not in _CACHE:
        _CACHE[key] = _build(nstep, with_b2)
    return _CACHE[key]


def _host_prep(coeffs, Wi1, bi1, Wi2, bi2, W1, b1, W2, b2, nstep: int):
    coeffs = np.asarray(coeffs, dtype=np.float32)
    a = coeffs[:, :, 0:8]
    b = coeffs[:, :, 8:16]
    c = coeffs[:, :, 16:24]
    d = coeffs[:, :, 24:32]

    X0 = a[:, 0]
    z0 = np.tanh(
        np.maximum(X0 @ Wi1 + bi1, 0.0).astype(np.float32) @ Wi2 + bi2
    ).astype(np.float32)

    # dX/dt at the three per-step sample classes, RK4 combine weights
    # folded in: class0 = dX(i) (k1 w=1), class1 = 2*dX(i+0.5) (k2+k3 w=2
    # each), class2 = dX(i+1) (k4 w=1).
    g = np.empty((B, nstep, 3, C_IN), dtype=np.float32)
    g[:, :, 0] = b[:, :nstep]
    g[:, :, 1] = 2.0 * b[:, :nstep] + 2.0 * c[:, :nstep] + 1.5 * d[:, :nstep]
    last = NSTEP - 1
    for i in range(nstep):
        if i < last:
            g[:, i, 2] = b[:, i + 1]
        else:
            g[:, i, 2] = b[:, i] + 2.0 * c[:, i] + 3.0 * d[:, i]

    tcols = np.empty((nstep, 3), dtype=np.float32)
    tcols[:, 0] = np.arange(nstep, dtype=np.float32)
    tcols[:, 1] = tcols[:, 0] + 0.5
    tcols[:, 2] = tcols[:, 0] + 1.0
    bias1 = (
        b1[None, None, :] + tcols[:, :, None] * W1[0][None, None, :]
    ).astype(np.float32)
    bias1 = bias1.reshape(nstep * 3, MLP_H).T.copy()

    w1z = np.ascontiguousarray(W1[1:], dtype=np.float32)  # (64, 128)
    # V tables: (S_j @ W1z) * alpha, laid out [p, (variant, j, m)]
    vfull = np.repeat(W1[1:], C_IN, axis=0).astype(np.float32)  # (512, 128)
    vt = np.stack([0.5 * vfull, 0.25 * vfull])  # (2, 512, 128)
    vt = vt.reshape(2, 4, 128, MLP_H).transpose(2, 0, 1, 3)
    vtab = np.ascontiguousarray(vt.reshape(128, 2 * NF), dtype=np.float16)
    # S pattern: group-of-8 partition sum, [p, (j, m)]
    q = np.arange(NF)
    sfull = (q[:, None] // C_IN == np.arange(HID)[None, :]).astype(np.float16)
    spat = np.ascontiguousarray(
        sfull.reshape(4, 128, HID).transpose(1, 0, 2).reshape(128, 4 * HID)
    )
    # b2 per-partition chunks [p, j] (only used when b2 != 0)
    b2t = np.ascontiguousarray(
        np.asarray(b2, np.float32).reshape(4, 128).T
    )

    v6 = (vfull / 6.0).reshape(4, 128, MLP_H).transpose(1, 0, 2)
    vt6 = np.ascontiguousarray(v6.reshape(128, NF), dtype=np.float16)

    shared = {
        "bias1": bias1,
        "w1z": w1z,
        "w1zh": np.ascontiguousarray(W1[1:], dtype=np.float16),
        "w2": np.ascontiguousarray(W2, dtype=np.float16),
        "vtab": vtab,
        "spat": spat,
        "b2t": b2t,
        "vt6": vt6,
    }
    in_maps = []
    for core in range(NCORES):
        sl = slice(core * BL, (core + 1) * BL)
        m = dict(shared)
        arr = g[sl].astype(np.float16)  # (BL, nstep, 3, 8)
        arr = arr.transpose(3, 1, 2, 0)  # (8, nstep, 3, BL)
        arr = np.tile(arr, (16, 1, 1, 1))  # (128, nstep, 3, BL); p%8 = c
        if _flags()[4]:
            arr = np.repeat(arr[:, :, :, None, :], 4, axis=3)
            m["dxr"] = np.ascontiguousarray(arr.reshape(128, nstep * 3 * 4 * BL))
        else:
            m["dxr"] = np.ascontiguousarray(arr.reshape(128, nstep * 3 * BL))
        m["z0t"] = np.ascontiguousarray(z0[sl].T)
        in_maps.append(m)
    return in_maps, z0


def kernel(coeffs, Wi1, bi1, Wi2, bi2, W1, b1, W2, b2, _nstep: int = NSTEP,
           _trace: bool = False):
    import sys
    import time as _time

    nstep = _nstep
    with_b2 = bool(np.any(np.asarray(b2)))
    nc = _get_nc(nstep, with_b2)
    in_maps, _ = _host_prep(
        coeffs, Wi1, bi1, Wi2, bi2, W1, b1, W2, b2, nstep
    )
    import os as _os

    v3 = _use_v2() and _os.environ.get("K_H0", "1") == "1"
    for m in in_maps:
        if _use_v2():
            m.pop("w1z", None)
        if not v3:
            m.pop("vt6", None)
    t0 = _time.time()
    res = run_bass_kernel_spmd(nc, in_maps, list(range(NCORES)), trace=_trace)
    print(f"[kernel] spmd run (compile+exec): {_time.time()-t0:.1f}s", file=sys.stderr)
    out = np.empty((B, nstep + 1, HID), dtype=np.float32)
    for core in range(NCORES):
        zs = res.results[core]["zs"].reshape(HID, nstep + 1, BL)
        out[core * BL : (core + 1) * BL] = zs.transpose(2, 1, 0)
    if _trace:
        kernel.last_results = res
    return out

